# revision 24
# baseline (speedup 1.0000x reference)
"""Causal self-attention (B=4, T=2048, C=1024, H=16, D=64) on 8 TRN2 NeuronCores.

Sharding: 4 batches x 2 head-groups (8 heads each). Core c handles batch c//2,
heads 8*(c%2) .. 8*(c%2)+7. Host pre-transposes x and slices/transposes the
weights (converted to bf16) so the device kernel needs no on-chip transposes:

  phase 1:  qkT[feat, T] = Wqk_g @ x^T   (features on partitions, t-chunk outer
            with coarse chained DMA admission in need order)
            V[t, vfeat]  = x @ Wv_g^T    (keys on partitions, + ones column)
            + the first q-chunk's attention duos interleaved between rounds,
            which spreads the exp load into the otherwise-idle ACT engine
  phase 2:  per head-pair interleaved: S^T[k, q] = K_h Q_h^T (bf16, k on
            partitions), P^T = exp(S^T/8) * causal masks (bf16),
            out^T[d|sum, q] = [V_h|1]^T P^T; normalization fused: DVE
            approx-reciprocal of the sum row (staged through SBUF), gpsimd
            partition-broadcast of 1/sum (library preloaded in phase 1), one
            tensor_mul reads PSUM and writes normalized bf16 attn^T
  phase 3:  y = attn^T.T @ Wp_g^T, interleaved between pairs as PE filler

All matmul inputs are bf16 (PSUM accumulation stays fp32), which enables
fast-weight-load and halves HBM/DVE traffic. PSUM: phase 1 holds 4 banks for
the projections + 2+2 for qc0's scores/outputs; afterwards the freed banks
double the score/output rotation. Each core returns a [2048, 1024] fp32
partial; the host sums the two head-group partials per batch.
"""

import numpy as np

T = 2048
N_CORES = 8

_CACHE = {}


def _build_module():
    from contextlib import ExitStack, nullcontext

    import concourse.tile as tile
    from concourse.tile_rust import add_dep_helper
    from concourse import bacc, mybir

    f32 = mybir.dt.float32
    bf16 = mybir.dt.bfloat16
    Exp = mybir.ActivationFunctionType.Exp
    Copy = mybir.ActivationFunctionType.Copy

    nc = bacc.Bacc("TRN2", target_bir_lowering=False, debug=False,
                   num_devices=N_CORES)

    # host-packed layouts: every DMA reads a fully contiguous DRAM range
    # (the naive [ci_full, f] layouts only give 256B-1KB runs -> ~15GB/s)
    xT_d = nc.dram_tensor("xT", (4, 128, 8 * 512), bf16, kind="ExternalInput").ap()
    wqkT_d = nc.dram_tensor("wqkT", (8, 128, 8 * 128), bf16,
                            kind="ExternalInput").ap()
    wvT_d = nc.dram_tensor("wvT", (128, 8, 512), bf16, kind="ExternalInput").ap()
    wpT_d = nc.dram_tensor("wpT", (128, 4, 1024), bf16, kind="ExternalInput").ap()
    mk_d = nc.dram_tensor("trimask", (128, 128), bf16, kind="ExternalInput").ap()
    y_d = nc.dram_tensor("y", (2048, 1024), f32, kind="ExternalOutput").ap()

    with tile.TileContext(nc) as tc, ExitStack() as ctx:
        pers = ctx.enter_context(tc.tile_pool(name="pers", bufs=1))
        sb_qT = pers.tile([128, 4, 2048], bf16, name="sb_qT")
        pq_tiles = [pers.tile([128, 512], bf16, name=f"pq{i}") for i in range(12)]
        sb_kT = pers.tile([128, 4, 2048], bf16, name="sb_kT")
        sb_v = pers.tile([128, 16, 520], bf16, name="sb_v")
        v_view = sb_v[:].rearrange("p t (h e) -> p t h e", e=65)
        sb_wpT = pers.tile([128, 4, 1024], bf16, name="sb_wpT")
        sb_trimask = pers.tile([128, 128], bf16, name="sb_trimask")
        sb_attnT = pers.tile([128, 4, 2048], bf16, name="sb_attnT")
        gp_src = pers.tile([1, 8], f32, name="gp_src")
        gp_dst = pers.tile([128, 8], f32, name="gp_dst")

        # pq0/1/4/5/8..11 serve even heads (zero band 64:128); pq2/3/6/7 odd
        PQ_EVEN = (0, 1, 4, 5, 8, 9, 10, 11)
        for i in range(12):
            o0, o1 = (64, 128) if i in PQ_EVEN else (0, 64)
            nc.vector.memset(pq_tiles[i][o0:o1, :], 0.0)
        # dummy broadcast: pulls the gpsimd library load (~7us) into phase 1
        # so the first softmax normalization doesn't pay it
        nc.vector.memset(gp_src[:], 0.0)
        nc.gpsimd.partition_broadcast(gp_dst[:], gp_src[:])

        # phase-2 pools created up front so qc0 attention can interleave
        # into phase 1. PSUM: 2 (scores) + 2 (attn out) + 4 (phase-1) banks;
        # after phase 1 the freed 4 banks become the B-side score/out pools.
        ps_sA = ctx.enter_context(tc.tile_pool(name="ps_sA", bufs=1,
                                               space="PSUM"))
        ps_oA = ctx.enter_context(tc.tile_pool(name="ps_oA", bufs=2,
                                               space="PSUM"))
        ps_s_pools = [ps_sA]
        ps_o_pools = [ps_oA]
        exp_pool = ctx.enter_context(tc.tile_pool(name="expp", bufs=8))
        norm_pool = ctx.enter_context(tc.tile_pool(name="normp", bufs=4))
        y_pool = ctx.enter_context(tc.tile_pool(name="yp", bufs=4))

        pss_n = [0]
        pso_n = [0]

        def alloc_pss():
            pool = ps_s_pools[pss_n[0] % len(ps_s_pools)]
            pss_n[0] += 1
            return pool.tile([128, 2, 512], f32, tag="pss",
                             name=f"pss{pss_n[0]}")

        def alloc_pso(shape):
            pool = ps_o_pools[pso_n[0] % len(ps_o_pools)]
            pso_n[0] += 1
            return pool.tile(shape, f32, tag="pso", name=f"pso{pso_n[0]}")

        def emit_proj(tblk, alt=False):
            for n in range(2):
                ysb = y_pool.tile([128, 512], f32, tag="ysb")
                pj = alloc_pso([128, 512])
                for ko in range(4):
                    nc.tensor.matmul(
                        pj[:],
                        lhsT=sb_attnT[:, ko, tblk * 128:(tblk + 1) * 128],
                        rhs=sb_wpT[:, ko, n * 512:(n + 1) * 512],
                        start=(ko == 0), stop=(ko == 3),
                    )
                if alt and n == 0:
                    nc.scalar.activation(ysb[:], pj[:], Copy)
                else:
                    nc.vector.tensor_copy(ysb[:], pj[:])
                nc.sync.dma_start(
                    y_d[tblk * 128:(tblk + 1) * 128, n * 512:(n + 1) * 512],
                    ysb[:])

        def norm_store(po, rr, p_, qc, tail=False):
            # normalize: sum row -> SBUF (recip from PSUM is broken on HW),
            # approx-reciprocal, gpsimd broadcast, then one fused mul reads
            # the PSUM attn rows and writes normalized bf16 attn^T.
            # Off the tail, run at low priority so the next duo's mask muls
            # win the DVE queue (a norm-first ordering stalls PE ~2us).
            att_slice = sb_attnT[rr:rr + 64, p_, qc * 512:(qc + 1) * 512]
            prio = (tc.high_priority(offset=300) if tail
                    else tc.high_priority(offset=-150))
            sums = norm_pool.tile([1, 512], f32, tag="sums")
            recip = norm_pool.tile([1, 512], f32, tag="recip")
            bcast = norm_pool.tile([128, 512], f32, tag="bcast")
            with prio:
                if tail:
                    # ACT is idle at the tail; free the DVE for the recip
                    nc.scalar.activation(sums[:], po[64:65, :], Copy)
                else:
                    nc.vector.tensor_copy(sums[:], po[64:65, :])
                nc.vector.reciprocal_approx_fast(out=recip[:], in_=sums[:])
                nc.gpsimd.partition_broadcast(bcast[:], recip[:])
                nc.vector.tensor_mul(att_slice, po[0:64, :], bcast[0:64, :])

        DUOS = ((0, 2), (4, 6), (1, 3), (5, 7))

        def emit_duo(qc, di, mid=None):
            nblk = 4 * qc + 4
            hA, hB = DUOS[di]
            rr = (hA % 2) * 64
            poA = alloc_pso([65, 512])
            poB = alloc_pso([65, 512])
            if qc == 1 and di in (0, 1):
                pqA, pqB = pq_tiles[8 + 2 * di], pq_tiles[9 + 2 * di]
            else:
                base = (0, 4, 2, 6)[di]
                pqA, pqB = pq_tiles[base], pq_tiles[base + 1]
            duo = ((hA, poA, pqA), (hB, poB, pqB))
            if qc > 0 and not (qc == 1 and di in (0, 1)):
                for h, po, pq in duo:
                    nc.vector.tensor_copy(
                        pq[rr:rr + 64, :],
                        sb_qT[rr:rr + 64, h // 2, qc * 512:(qc + 1) * 512])
            for jg in range(nblk // 2):
                # all scores of the duo first: the partner's scores hide
                # each head's exp latency before its PV consumes it.
                # Diagonal pairs skip the below-diagonal score columns
                # (never read); the i01 pair swaps PSUM slots so the live
                # region is one contiguous flat range for a single exp.
                diag0 = jg * 2 == 4 * qc        # blocks i=0,1
                diag2 = jg * 2 == 4 * qc + 2    # blocks i=2,3
                ets = []
                for h, po, pq in duo:
                    p_ = h // 2
                    pss = alloc_pss()
                    for jj in range(2):
                        j = jg * 2 + jj
                        slot = 1 - jj if diag0 else jj
                        ls = 128 * jj if diag0 else (256 if diag2 else 0)
                        nc.tensor.matmul(
                            pss[:, slot, ls:512],
                            lhsT=sb_kT[:, p_, j * 128:(j + 1) * 128],
                            rhs=pq[:, ls:512],
                            start=True, stop=True,
                        )
                    et = exp_pool.tile([128, 2, 512], bf16, tag="expT")
                    if diag2:
                        nc.scalar.activation(et[:, :, 256:512],
                                             pss[:, :, 256:512],
                                             Exp, scale=0.125)
                    elif diag0:
                        ef = et[:].rearrange("p a b -> p (a b)")
                        pf = pss[:].rearrange("p a b -> p (a b)")
                        nc.scalar.activation(ef[:, 128:1024], pf[:, 128:1024],
                                             Exp, scale=0.125)
                    else:
                        nc.scalar.activation(et[:], pss[:], Exp, scale=0.125)
                    ets.append(et)
                for (h, po, pq), et in zip(duo, ets):
                    for jj in range(2):
                        j = jg * 2 + jj
                        slot = 1 - jj if diag0 else jj
                        lo = 0
                        if j >= 4 * qc:
                            i = j - 4 * qc
                            lo = i * 128
                            nc.vector.tensor_mul(
                                et[:, slot, lo:lo + 128],
                                et[:, slot, lo:lo + 128], sb_trimask[:])
                        nc.tensor.matmul(
                            po[:, lo:512],
                            lhsT=v_view[:, j, h, :],
                            rhs=et[:, slot, lo:512],
                            start=(j == 0), stop=(j == nblk - 1),
                        )
            if mid is not None:
                mid()
            for h, po, pq in duo:
                norm_store(po, (h % 2) * 64, h // 2, qc,
                           tail=(qc == 3 and di == 3))
            if qc > 0:
                emit_proj((qc - 1) * 4 + di)

        # ---------------- phase 1: qkv projections (+ qc0 attention) -------
        with ExitStack() as p1:
            ph1 = p1.enter_context(tc.tile_pool(name="ph1", bufs=1))
            ps_big = p1.enter_context(tc.tile_pool(name="ps_big", bufs=4,
                                                   space="PSUM"))
            wqk_pool = p1.enter_context(tc.tile_pool(name="wqk", bufs=8))
            xt_pool = p1.enter_context(tc.tile_pool(name="xt", bufs=2))

            wqkT_r = wqkT_d.rearrange("fb ci (co f) -> fb ci co f", f=128)
            xT_r = xT_d.rearrange("tc ci (co t) -> tc ci co t", t=512)

            chain_prev = [None]

            def chain(d):
                if chain_prev[0] is not None:
                    add_dep_helper(d.ins, chain_prev[0].ins, sync=False,
                                   reason="adm chain")
                chain_prev[0] = d
                return d

            # K features first, then Q
            fbs = [4, 5, 6, 7, 0, 1, 2, 3]
            wtiles = {fb: wqk_pool.tile([128, 8, 128], bf16, tag="wqk",
                                        name=f"wt{fb}") for fb in fbs}
            sb_wvT = ph1.tile([128, 8, 512], bf16, name="sb_wvT")
            xchunk = xt_pool.tile([128, 8, 512], bf16, tag="xt", name="xc0")
            # admission order = need order (descriptors round-robin across the
            # 16 DMA engines in trigger order, so early triggers finish first);
            # the head is split fine so the first matmuls start ~9us in
            chain(nc.sync.dma_start(wtiles[4][:, 0:2, :], wqkT_r[4, :, 0:2, :]))
            chain(nc.sync.dma_start(xchunk[:, 0:2, :], xT_r[0, :, 0:2, :]))
            chain(nc.sync.dma_start(wtiles[4][:, 2:5, :], wqkT_r[4, :, 2:5, :]))
            chain(nc.sync.dma_start(xchunk[:, 2:5, :], xT_r[0, :, 2:5, :]))
            chain(nc.sync.dma_start(wtiles[4][:, 5:8, :], wqkT_r[4, :, 5:8, :]))
            chain(nc.sync.dma_start(xchunk[:, 5:8, :], xT_r[0, :, 5:8, :]))
            chain(nc.sync.dma_start(wtiles[5][:], wqkT_r[5]))
            for fb in (6, 7):
                chain(nc.sync.dma_start(wtiles[fb][:], wqkT_r[fb]))
            chain(nc.sync.dma_start(sb_trimask[:], mk_d[:]))
            for fb in (0, 1, 2, 3):
                chain(nc.sync.dma_start(wtiles[fb][:], wqkT_r[fb]))
            chain(nc.sync.dma_start(sb_wvT[:], wvT_d[:]))
            chain(nc.sync.dma_start(sb_wpT[:], wpT_d[:]))

            # ones column of sb_v via exp(0)=1 — also warms the ACT exp table
            zeros = ph1.tile([128, 128], f32, name="zeros")
            nc.vector.memset(zeros[:], 0.0)
            nc.scalar.activation(
                v_view[:, :, :, 64:65],
                zeros[:].rearrange("p (a b c) -> p a b c", a=16, b=8),
                Exp,
            )

            for tci in range(4):
                if tci < 3:
                    nxt = xt_pool.tile([128, 8, 512], bf16, tag="xt",
                                       name=f"xc{tci + 1}")
                    chain(nc.sync.dma_start(nxt[:], xT_r[tci + 1]))
                for fb in fbs:
                    dst, pblk = (sb_kT, fb - 4) if fb >= 4 else (sb_qT, fb)
                    ps = ps_big.tile([128, 512], f32, tag="psb")
                    for co in range(8):
                        nc.tensor.matmul(
                            ps[:],
                            lhsT=wtiles[fb][:, co, :],
                            rhs=xchunk[:, co, :],
                            start=(co == 0), stop=(co == 7),
                        )
                    if tci >= 2 and fb % 2 == 0:
                        nc.scalar.activation(
                            dst[:, pblk, tci * 512:(tci + 1) * 512], ps[:],
                            Copy)
                    else:
                        nc.vector.tensor_copy(
                            dst[:, pblk, tci * 512:(tci + 1) * 512], ps[:])
                    if tci == 0 and fb == 3:
                        # prefill padded-q for all of qc0
                        for pi, h in ((0, 0), (1, 2), (4, 4), (5, 6),
                                      (2, 1), (3, 3), (6, 5), (7, 7)):
                            rr0 = (h % 2) * 64
                            nc.vector.tensor_copy(
                                pq_tiles[pi][rr0:rr0 + 64, :],
                                sb_qT[rr0:rr0 + 64, h // 2, 0:512])
                    if tci == 1 and fb == 3:
                        # prefill qc1's even duos too (ramp relief)
                        for pi, h in ((8, 0), (9, 2), (10, 4), (11, 6)):
                            nc.vector.tensor_copy(
                                pq_tiles[pi][0:64, :],
                                sb_qT[0:64, h // 2, 512:1024])
                if tci == 1:
                    emit_duo(0, 0)
                elif tci == 2:
                    emit_duo(0, 2)
                elif tci == 3:
                    emit_duo(1, 0)
                for tb in range(4):
                    tblk = tci * 4 + tb
                    ps = ps_big.tile([128, 512], f32, tag="psb")
                    for co in range(8):
                        nc.tensor.matmul(
                            ps[:],
                            lhsT=xchunk[:, co, tb * 128:(tb + 1) * 128],
                            rhs=sb_wvT[:, co, :],
                            start=(co == 0), stop=(co == 7),
                        )
                    lowprio = (tc.high_priority(offset=-400) if tci == 3
                               else nullcontext())
                    with lowprio:
                        if tci >= 2 and tb % 2 == 0:
                            nc.scalar.activation(
                                v_view[:, tblk, :, 0:64],
                                ps[:].rearrange("p (h d) -> p h d", d=64),
                                Copy)
                        else:
                            nc.vector.tensor_copy(
                                v_view[:, tblk, :, 0:64],
                                ps[:].rearrange("p (h d) -> p h d", d=64),
                            )
                if tci == 1:
                    emit_duo(0, 1)
                elif tci == 2:
                    emit_duo(0, 3)
                elif tci == 3:
                    emit_duo(1, 1)
                if tci < 3:
                    xchunk = nxt

        # ---------------- phase 2 + 3: attention + proj ----------------
        # phase 1's 4 PSUM banks are free now: add the B-side pools so the
        # score/out rotations double (allocators alternate A/B)
        ps_s_pools.append(ctx.enter_context(
            tc.tile_pool(name="ps_sB", bufs=1, space="PSUM")))
        ps_o_pools.append(ctx.enter_context(
            tc.tile_pool(name="ps_oB", bufs=2, space="PSUM")))

        for qc in range(1, 4):
            # duo interleave: second head fills the first head's exp latency
            # (qc1 duos 0,1 already ran inside phase 1)
            for di in range(2 if qc == 1 else 0, 4):
                if qc == 3 and di == 3:
                    # keep PE fed while the last duo's norm chain drains:
                    # tblk 12's ko0/ko1 matmuls (heads 0-3, long ready) fill
                    # the gap; ko2/ko3 follow after the norm
                    fpjs = []

                    def mid():
                        for n in range(2):
                            pj = alloc_pso([128, 512])
                            for ko in (0, 1):
                                nc.tensor.matmul(
                                    pj[:],
                                    lhsT=sb_attnT[:, ko, 12 * 128:13 * 128],
                                    rhs=sb_wpT[:, ko, n * 512:(n + 1) * 512],
                                    start=(ko == 0), stop=False,
                                )
                            fpjs.append((n, pj))

                    emit_duo(3, 3, mid=mid)
                    for n, pj in fpjs:
                        for ko in (2, 3):
                            nc.tensor.matmul(
                                pj[:],
                                lhsT=sb_attnT[:, ko, 12 * 128:13 * 128],
                                rhs=sb_wpT[:, ko, n * 512:(n + 1) * 512],
                                start=False, stop=(ko == 3),
                            )
                        ysb = y_pool.tile([128, 512], f32, tag="ysb")
                        if n == 0:
                            nc.scalar.activation(ysb[:], pj[:], Copy)
                        else:
                            nc.vector.tensor_copy(ysb[:], pj[:])
                        nc.sync.dma_start(
                            y_d[12 * 128:13 * 128, n * 512:(n + 1) * 512],
                            ysb[:])
                else:
                    emit_duo(qc, di)
            if qc == 3:
                for tblk in range(13, 16):
                    emit_proj(tblk, alt=True)

    nc.compile()
    return nc


def _get_module():
    if "nc" not in _CACHE:
        _CACHE["nc"] = _build_module()
    return _CACHE["nc"]


def _make_trimask():
    # trimask[kk, q] = 1 iff q >= kk (diagonal 128x128 block)
    q = np.arange(128)[None, :]
    kk = np.arange(128)[:, None]
    return (q >= kk).astype(np.float32)


def make_in_maps(x, W_qkv, W_proj):
    import ml_dtypes

    bf16 = ml_dtypes.bfloat16
    x = np.asarray(x, dtype=np.float32)
    W_qkv = np.asarray(W_qkv, dtype=np.float32)
    W_proj = np.asarray(W_proj, dtype=np.float32)
    trimask = _make_trimask().astype(bf16)
    in_maps = []
    for c in range(N_CORES):
        b, g = c // 2, c % 2
        s = 512 * g
        wqk = np.concatenate([W_qkv[s:s + 512], W_qkv[1024 + s:1024 + s + 512]], 0)
        # packed layouts (see _build_module): contiguous per DMA
        xt = x[b].T.reshape(8, 128, 4, 512).transpose(2, 1, 0, 3)   # tc ci co t
        wqkt = wqk.T.reshape(8, 128, 8, 128).transpose(2, 1, 0, 3)  # fb ci co f
        wvt = W_qkv[2048 + s:2048 + s + 512].T.reshape(8, 128, 512)
        wvt = wvt.transpose(1, 0, 2)                                # ci co f
        wpt = W_proj[:, s:s + 512].T.reshape(4, 128, 1024)
        wpt = wpt.transpose(1, 0, 2)                                # ki ko n
        in_maps.append({
            "xT": np.ascontiguousarray(xt.reshape(4, 128, 8 * 512)).astype(bf16),
            "wqkT": np.ascontiguousarray(wqkt.reshape(8, 128, 8 * 128)).astype(bf16),
            "wvT": np.ascontiguousarray(wvt).astype(bf16),
            "wpT": np.ascontiguousarray(wpt).astype(bf16),
            "trimask": trimask,
        })
    return in_maps


def run(x, W_qkv, W_proj, trace=False):
    """Returns (y_full [4,2048,1024], BassKernelResults)."""
    from concourse import bass_utils

    nc = _get_module()
    in_maps = make_in_maps(x, W_qkv, W_proj)
    res = bass_utils.run_bass_kernel_spmd(
        nc, in_maps, core_ids=list(range(N_CORES)), trace=trace)
    y = np.zeros((4, T, 1024), np.float32)
    for b in range(4):
        y[b] = res.results[2 * b]["y"] + res.results[2 * b + 1]["y"]
    return y, res


def kernel(x, W_qkv, W_proj):
    y, _ = run(x, W_qkv, W_proj, trace=False)
    return y


# revision 30
# speedup vs baseline: 1.0041x; 1.0041x over previous
"""Causal self-attention (B=4, T=2048, C=1024, H=16, D=64) on 8 TRN2 NeuronCores.

Sharding: 4 batches x 2 head-groups (8 heads each). Core c handles batch c//2,
heads 8*(c%2) .. 8*(c%2)+7. Host pre-transposes x and slices/transposes the
weights (converted to bf16) so the device kernel needs no on-chip transposes:

  phase 1:  qkT[feat, T] = Wqk_g @ x^T   (features on partitions, t-chunk outer
            with coarse chained DMA admission in need order)
            V[t, vfeat]  = x @ Wv_g^T    (keys on partitions, + ones column)
            + the first q-chunk's attention duos interleaved between rounds,
            which spreads the exp load into the otherwise-idle ACT engine
  phase 2:  per head-pair interleaved: S^T[k, q] = K_h Q_h^T (bf16, k on
            partitions), P^T = exp(S^T/8) * causal masks (bf16),
            out^T[d|sum, q] = [V_h|1]^T P^T; normalization fused: DVE
            approx-reciprocal of the sum row (staged through SBUF), gpsimd
            partition-broadcast of 1/sum (library preloaded in phase 1), one
            tensor_mul reads PSUM and writes normalized bf16 attn^T
  phase 3:  y = attn^T.T @ Wp_g^T, interleaved between pairs as PE filler

All matmul inputs are bf16 (PSUM accumulation stays fp32), which enables
fast-weight-load and halves HBM/DVE traffic. PSUM: phase 1 holds 4 banks for
the projections + 2+2 for qc0's scores/outputs; afterwards the freed banks
double the score/output rotation. Each core returns a [2048, 1024] fp32
partial; the host sums the two head-group partials per batch.
"""

import numpy as np

T = 2048
N_CORES = 8

_CACHE = {}


def _build_module():
    from contextlib import ExitStack, nullcontext

    import concourse.tile as tile
    from concourse.tile_rust import add_dep_helper
    from concourse import bacc, mybir

    f32 = mybir.dt.float32
    bf16 = mybir.dt.bfloat16
    Exp = mybir.ActivationFunctionType.Exp
    Copy = mybir.ActivationFunctionType.Copy

    nc = bacc.Bacc("TRN2", target_bir_lowering=False, debug=False,
                   num_devices=N_CORES)

    # host-packed layouts: every DMA reads a fully contiguous DRAM range
    # (the naive [ci_full, f] layouts only give 256B-1KB runs -> ~15GB/s)
    xT_d = nc.dram_tensor("xT", (4, 128, 8 * 512), bf16, kind="ExternalInput").ap()
    wqkT_d = nc.dram_tensor("wqkT", (8, 128, 8 * 128), bf16,
                            kind="ExternalInput").ap()
    wvT_d = nc.dram_tensor("wvT", (128, 8, 512), bf16, kind="ExternalInput").ap()
    wpT_d = nc.dram_tensor("wpT", (128, 4, 1024), bf16, kind="ExternalInput").ap()
    mk_d = nc.dram_tensor("trimask", (128, 128), bf16, kind="ExternalInput").ap()
    y_d = nc.dram_tensor("y", (2048, 1024), f32, kind="ExternalOutput").ap()

    with tile.TileContext(nc) as tc, ExitStack() as ctx:
        pers = ctx.enter_context(tc.tile_pool(name="pers", bufs=1))
        sb_qT = pers.tile([128, 4, 2048], bf16, name="sb_qT")
        pq_tiles = [pers.tile([128, 512], bf16, name=f"pq{i}") for i in range(12)]
        sb_kT = pers.tile([128, 4, 2048], bf16, name="sb_kT")
        sb_v = pers.tile([128, 16, 520], bf16, name="sb_v")
        v_view = sb_v[:].rearrange("p t (h e) -> p t h e", e=65)
        sb_wpT = pers.tile([128, 4, 1024], bf16, name="sb_wpT")
        sb_trimask = pers.tile([128, 128], bf16, name="sb_trimask")
        sb_attnT = pers.tile([128, 4, 2048], bf16, name="sb_attnT")
        gp_src = pers.tile([1, 8], f32, name="gp_src")
        gp_dst = pers.tile([128, 8], f32, name="gp_dst")

        # pq0/1/4/5/8..11 serve even heads (zero band 64:128); pq2/3/6/7 odd
        PQ_EVEN = (0, 1, 4, 5, 8, 9, 10, 11)
        for i in range(12):
            o0, o1 = (64, 128) if i in PQ_EVEN else (0, 64)
            nc.vector.memset(pq_tiles[i][o0:o1, :], 0.0)
        # dummy broadcast: pulls the gpsimd library load (~7us) into phase 1
        # so the first softmax normalization doesn't pay it
        nc.vector.memset(gp_src[:], 0.0)
        nc.gpsimd.partition_broadcast(gp_dst[:], gp_src[:])

        # phase-2 pools created up front so qc0 attention can interleave
        # into phase 1. PSUM: 2 (scores) + 2 (attn out) + 4 (phase-1) banks;
        # after phase 1 the freed 4 banks become the B-side score/out pools.
        ps_sA = ctx.enter_context(tc.tile_pool(name="ps_sA", bufs=1,
                                               space="PSUM"))
        ps_oA = ctx.enter_context(tc.tile_pool(name="ps_oA", bufs=2,
                                               space="PSUM"))
        ps_s_pools = [ps_sA]
        ps_o_pools = [ps_oA]
        exp_pool = ctx.enter_context(tc.tile_pool(name="expp", bufs=8))
        norm_pool = ctx.enter_context(tc.tile_pool(name="normp", bufs=4))
        y_pool = ctx.enter_context(tc.tile_pool(name="yp", bufs=4))

        pss_n = [0]
        pso_n = [0]

        def alloc_pss():
            pool = ps_s_pools[pss_n[0] % len(ps_s_pools)]
            pss_n[0] += 1
            return pool.tile([128, 2, 512], f32, tag="pss",
                             name=f"pss{pss_n[0]}")

        def alloc_pso(shape):
            pool = ps_o_pools[pso_n[0] % len(ps_o_pools)]
            pso_n[0] += 1
            return pool.tile(shape, f32, tag="pso", name=f"pso{pso_n[0]}")

        def emit_proj(tblk, alt=False):
            for n in range(2):
                ysb = y_pool.tile([128, 512], f32, tag="ysb")
                pj = alloc_pso([128, 512])
                for ko in range(4):
                    nc.tensor.matmul(
                        pj[:],
                        lhsT=sb_attnT[:, ko, tblk * 128:(tblk + 1) * 128],
                        rhs=sb_wpT[:, ko, n * 512:(n + 1) * 512],
                        start=(ko == 0), stop=(ko == 3),
                    )
                if alt and n == 0:
                    nc.scalar.activation(ysb[:], pj[:], Copy)
                else:
                    nc.vector.tensor_copy(ysb[:], pj[:])
                nc.sync.dma_start(
                    y_d[tblk * 128:(tblk + 1) * 128, n * 512:(n + 1) * 512],
                    ysb[:])

        def norm_store(po, rr, p_, qc, tail=False):
            # normalize: sum row -> SBUF (recip from PSUM is broken on HW),
            # approx-reciprocal, gpsimd broadcast, then one fused mul reads
            # the PSUM attn rows and writes normalized bf16 attn^T.
            # Off the tail, run at low priority so the next duo's mask muls
            # win the DVE queue (a norm-first ordering stalls PE ~2us).
            att_slice = sb_attnT[rr:rr + 64, p_, qc * 512:(qc + 1) * 512]
            prio = (tc.high_priority(offset=300) if tail
                    else tc.high_priority(offset=-150))
            sums = norm_pool.tile([1, 512], f32, tag="sums")
            recip = norm_pool.tile([1, 512], f32, tag="recip")
            bcast = norm_pool.tile([128, 512], f32, tag="bcast")
            with prio:
                if tail:
                    # ACT is idle at the tail; free the DVE for the recip
                    nc.scalar.activation(sums[:], po[64:65, :], Copy)
                else:
                    nc.vector.tensor_copy(sums[:], po[64:65, :])
                nc.vector.reciprocal_approx_fast(out=recip[:], in_=sums[:])
                nc.gpsimd.partition_broadcast(bcast[:], recip[:])
                nc.vector.tensor_mul(att_slice, po[0:64, :], bcast[0:64, :])

        DUOS = ((0, 2), (4, 6), (1, 3), (5, 7))

        def emit_duo(qc, di, mid=None, proj=True):
            nblk = 4 * qc + 4
            hA, hB = DUOS[di]
            rr = (hA % 2) * 64
            poA = alloc_pso([65, 512])
            poB = alloc_pso([65, 512])
            if qc == 1 and di in (0, 1):
                pqA, pqB = pq_tiles[8 + 2 * di], pq_tiles[9 + 2 * di]
            else:
                base = (0, 4, 2, 6)[di]
                pqA, pqB = pq_tiles[base], pq_tiles[base + 1]
            duo = ((hA, poA, pqA), (hB, poB, pqB))
            if qc > 0 and not (qc == 1 and di in (0, 1)):
                for h, po, pq in duo:
                    nc.vector.tensor_copy(
                        pq[rr:rr + 64, :],
                        sb_qT[rr:rr + 64, h // 2, qc * 512:(qc + 1) * 512])
            for jg in range(nblk // 2):
                # all scores of the duo first: the partner's scores hide
                # each head's exp latency before its PV consumes it.
                # Diagonal pairs skip the below-diagonal score columns
                # (never read); the i01 pair swaps PSUM slots so the live
                # region is one contiguous flat range for a single exp.
                diag0 = jg * 2 == 4 * qc        # blocks i=0,1
                diag2 = jg * 2 == 4 * qc + 2    # blocks i=2,3
                ets = []
                for h, po, pq in duo:
                    p_ = h // 2
                    pss = alloc_pss()
                    for jj in range(2):
                        j = jg * 2 + jj
                        slot = 1 - jj if diag0 else jj
                        ls = 128 * jj if diag0 else (256 if diag2 else 0)
                        nc.tensor.matmul(
                            pss[:, slot, ls:512],
                            lhsT=sb_kT[:, p_, j * 128:(j + 1) * 128],
                            rhs=pq[:, ls:512],
                            start=True, stop=True,
                        )
                    et = exp_pool.tile([128, 2, 512], bf16, tag="expT")
                    if diag2:
                        nc.scalar.activation(et[:, :, 256:512],
                                             pss[:, :, 256:512],
                                             Exp, scale=0.125)
                    elif diag0:
                        ef = et[:].rearrange("p a b -> p (a b)")
                        pf = pss[:].rearrange("p a b -> p (a b)")
                        nc.scalar.activation(ef[:, 128:1024], pf[:, 128:1024],
                                             Exp, scale=0.125)
                    else:
                        nc.scalar.activation(et[:], pss[:], Exp, scale=0.125)
                    ets.append(et)
                for (h, po, pq), et in zip(duo, ets):
                    for jj in range(2):
                        j = jg * 2 + jj
                        slot = 1 - jj if diag0 else jj
                        lo = 0
                        if j >= 4 * qc:
                            i = j - 4 * qc
                            lo = i * 128
                            nc.vector.tensor_mul(
                                et[:, slot, lo:lo + 128],
                                et[:, slot, lo:lo + 128], sb_trimask[:])
                        nc.tensor.matmul(
                            po[:, lo:512],
                            lhsT=v_view[:, j, h, :],
                            rhs=et[:, slot, lo:512],
                            start=(j == 0), stop=(j == nblk - 1),
                        )
            if mid is not None:
                mid()
            for h, po, pq in duo:
                norm_store(po, (h % 2) * 64, h // 2, qc,
                           tail=(qc == 3 and di == 3))
            if qc > 0 and proj:
                emit_proj((qc - 1) * 4 + di)

        # ---------------- phase 1: qkv projections (+ qc0 attention) -------
        with ExitStack() as p1:
            ph1 = p1.enter_context(tc.tile_pool(name="ph1", bufs=1))
            ps_big = p1.enter_context(tc.tile_pool(name="ps_big", bufs=4,
                                                   space="PSUM"))
            wqk_pool = p1.enter_context(tc.tile_pool(name="wqk", bufs=8))
            xt_pool = p1.enter_context(tc.tile_pool(name="xt", bufs=2))

            wqkT_r = wqkT_d.rearrange("fb ci (co f) -> fb ci co f", f=128)
            xT_r = xT_d.rearrange("tc ci (co t) -> tc ci co t", t=512)

            chain_prev = [None]

            def chain(d):
                if chain_prev[0] is not None:
                    add_dep_helper(d.ins, chain_prev[0].ins, sync=False,
                                   reason="adm chain")
                chain_prev[0] = d
                return d

            # K features first, then Q
            fbs = [4, 5, 6, 7, 0, 1, 2, 3]
            wtiles = {fb: wqk_pool.tile([128, 8, 128], bf16, tag="wqk",
                                        name=f"wt{fb}") for fb in fbs}
            sb_wvT = ph1.tile([128, 8, 512], bf16, name="sb_wvT")
            xchunk = xt_pool.tile([128, 8, 512], bf16, tag="xt", name="xc0")
            # admission order = need order (descriptors round-robin across the
            # 16 DMA engines in trigger order, so early triggers finish first)
            chain(nc.sync.dma_start(wtiles[4][:], wqkT_r[4]))
            chain(nc.sync.dma_start(xchunk[:, 0:4, :], xT_r[0, :, 0:4, :]))
            chain(nc.sync.dma_start(wtiles[5][:], wqkT_r[5]))
            chain(nc.sync.dma_start(xchunk[:, 4:8, :], xT_r[0, :, 4:8, :]))
            for fb in (6, 7):
                chain(nc.sync.dma_start(wtiles[fb][:], wqkT_r[fb]))
            chain(nc.sync.dma_start(sb_trimask[:], mk_d[:]))
            for fb in (0, 1, 2, 3):
                chain(nc.sync.dma_start(wtiles[fb][:], wqkT_r[fb]))
            chain(nc.sync.dma_start(sb_wvT[:], wvT_d[:]))
            chain(nc.sync.dma_start(sb_wpT[:], wpT_d[:]))

            # ones column of sb_v via exp(0)=1 — also warms the ACT exp table
            zeros = ph1.tile([128, 128], f32, name="zeros")
            nc.vector.memset(zeros[:], 0.0)
            nc.scalar.activation(
                v_view[:, :, :, 64:65],
                zeros[:].rearrange("p (a b c) -> p a b c", a=16, b=8),
                Exp,
            )

            for tci in range(4):
                if tci < 3:
                    nxt = xt_pool.tile([128, 8, 512], bf16, tag="xt",
                                       name=f"xc{tci + 1}")
                    chain(nc.sync.dma_start(nxt[:], xT_r[tci + 1]))
                for fb in fbs:
                    dst, pblk = (sb_kT, fb - 4) if fb >= 4 else (sb_qT, fb)
                    ps = ps_big.tile([128, 512], f32, tag="psb")
                    for co in range(8):
                        nc.tensor.matmul(
                            ps[:],
                            lhsT=wtiles[fb][:, co, :],
                            rhs=xchunk[:, co, :],
                            start=(co == 0), stop=(co == 7),
                        )
                    if tci >= 2 and fb % 2 == 0:
                        nc.scalar.activation(
                            dst[:, pblk, tci * 512:(tci + 1) * 512], ps[:],
                            Copy)
                    else:
                        nc.vector.tensor_copy(
                            dst[:, pblk, tci * 512:(tci + 1) * 512], ps[:])
                    if tci == 0 and fb == 3:
                        # prefill padded-q for all of qc0
                        for pi, h in ((0, 0), (1, 2), (4, 4), (5, 6),
                                      (2, 1), (3, 3), (6, 5), (7, 7)):
                            rr0 = (h % 2) * 64
                            nc.vector.tensor_copy(
                                pq_tiles[pi][rr0:rr0 + 64, :],
                                sb_qT[rr0:rr0 + 64, h // 2, 0:512])
                    if tci == 1 and fb == 3:
                        # prefill qc1's even duos too (ramp relief)
                        for pi, h in ((8, 0), (9, 2), (10, 4), (11, 6)):
                            nc.vector.tensor_copy(
                                pq_tiles[pi][0:64, :],
                                sb_qT[0:64, h // 2, 512:1024])
                if tci == 1:
                    emit_duo(0, 0)
                elif tci == 2:
                    emit_duo(0, 2)
                elif tci == 3:
                    # no proj: the pj allocations would cascade-stall the
                    # 2-deep phase-1 PSUM out-ring; deferred to phase 2
                    emit_duo(1, 0, proj=False)
                for tb in range(4):
                    tblk = tci * 4 + tb
                    ps = ps_big.tile([128, 512], f32, tag="psb")
                    for co in range(8):
                        nc.tensor.matmul(
                            ps[:],
                            lhsT=xchunk[:, co, tb * 128:(tb + 1) * 128],
                            rhs=sb_wvT[:, co, :],
                            start=(co == 0), stop=(co == 7),
                        )
                    lowprio = (tc.high_priority(offset=-400) if tci == 3
                               else nullcontext())
                    with lowprio:
                        if tci >= 2 and tb % 2 == 0:
                            nc.scalar.activation(
                                v_view[:, tblk, :, 0:64],
                                ps[:].rearrange("p (h d) -> p h d", d=64),
                                Copy)
                        else:
                            nc.vector.tensor_copy(
                                v_view[:, tblk, :, 0:64],
                                ps[:].rearrange("p (h d) -> p h d", d=64),
                            )
                    if tci == 3 and tb == 1:
                        # the trailing v-groups queue behind this duo's
                        # scores and fill its first exp bubble
                        emit_duo(1, 1, proj=False)
                if tci == 1:
                    emit_duo(0, 1)
                elif tci == 2:
                    emit_duo(0, 3)
                if tci < 3:
                    xchunk = nxt

        # ---------------- phase 2 + 3: attention + proj ----------------
        # phase 1's 4 PSUM banks are free now: add the B-side pools so the
        # score/out rotations double (allocators alternate A/B)
        ps_s_pools.append(ctx.enter_context(
            tc.tile_pool(name="ps_sB", bufs=1, space="PSUM")))
        ps_o_pools.append(ctx.enter_context(
            tc.tile_pool(name="ps_oB", bufs=2, space="PSUM")))

        # projections for tblks 0,1 were deferred out of phase 1
        emit_proj(0)
        emit_proj(1)
        for qc in range(1, 4):
            # duo interleave: second head fills the first head's exp latency
            # (qc1 duos 0,1 already ran inside phase 1)
            for di in range(2 if qc == 1 else 0, 4):
                if qc == 3 and di == 3:
                    # keep PE fed while the last duo's norm chain drains:
                    # tblks 12,13 run through the freed score-PSUM rings
                    # ([128,2,512] = both column halves); their ko0/ko1
                    # matmuls (heads 0-3, long ready) fill the gap, ko2/ko3
                    # follow after the norm
                    fpjs = []

                    def mid():
                        for tblk in (12, 13):
                            pj = alloc_pss()
                            for n in range(2):
                                for ko in (0, 1):
                                    nc.tensor.matmul(
                                        pj[:, n, :],
                                        lhsT=sb_attnT[:, ko,
                                                      tblk * 128:(tblk + 1) * 128],
                                        rhs=sb_wpT[:, ko,
                                                   n * 512:(n + 1) * 512],
                                        start=(ko == 0), stop=False,
                                    )
                            fpjs.append((tblk, pj))

                    emit_duo(3, 3, mid=mid)
                    for tblk, pj in fpjs:
                        for n in range(2):
                            for ko in (2, 3):
                                nc.tensor.matmul(
                                    pj[:, n, :],
                                    lhsT=sb_attnT[:, ko,
                                                  tblk * 128:(tblk + 1) * 128],
                                    rhs=sb_wpT[:, ko, n * 512:(n + 1) * 512],
                                    start=False, stop=(ko == 3),
                                )
                            ysb = y_pool.tile([128, 512], f32, tag="ysb")
                            if n == 0:
                                nc.scalar.activation(ysb[:], pj[:, n, :], Copy)
                            else:
                                nc.vector.tensor_copy(ysb[:], pj[:, n, :])
                            nc.sync.dma_start(
                                y_d[tblk * 128:(tblk + 1) * 128,
                                    n * 512:(n + 1) * 512],
                                ysb[:])
                else:
                    emit_duo(qc, di)
            if qc == 3:
                for tblk in range(14, 16):
                    emit_proj(tblk, alt=True)

    nc.compile()
    return nc


def _get_module():
    if "nc" not in _CACHE:
        _CACHE["nc"] = _build_module()
    return _CACHE["nc"]


def _make_trimask():
    # trimask[kk, q] = 1 iff q >= kk (diagonal 128x128 block)
    q = np.arange(128)[None, :]
    kk = np.arange(128)[:, None]
    return (q >= kk).astype(np.float32)


def make_in_maps(x, W_qkv, W_proj):
    import ml_dtypes

    bf16 = ml_dtypes.bfloat16
    x = np.asarray(x, dtype=np.float32)
    W_qkv = np.asarray(W_qkv, dtype=np.float32)
    W_proj = np.asarray(W_proj, dtype=np.float32)
    trimask = _make_trimask().astype(bf16)
    in_maps = []
    for c in range(N_CORES):
        b, g = c // 2, c % 2
        s = 512 * g
        wqk = np.concatenate([W_qkv[s:s + 512], W_qkv[1024 + s:1024 + s + 512]], 0)
        # packed layouts (see _build_module): contiguous per DMA
        xt = x[b].T.reshape(8, 128, 4, 512).transpose(2, 1, 0, 3)   # tc ci co t
        wqkt = wqk.T.reshape(8, 128, 8, 128).transpose(2, 1, 0, 3)  # fb ci co f
        wvt = W_qkv[2048 + s:2048 + s + 512].T.reshape(8, 128, 512)
        wvt = wvt.transpose(1, 0, 2)                                # ci co f
        wpt = W_proj[:, s:s + 512].T.reshape(4, 128, 1024)
        wpt = wpt.transpose(1, 0, 2)                                # ki ko n
        in_maps.append({
            "xT": np.ascontiguousarray(xt.reshape(4, 128, 8 * 512)).astype(bf16),
            "wqkT": np.ascontiguousarray(wqkt.reshape(8, 128, 8 * 128)).astype(bf16),
            "wvT": np.ascontiguousarray(wvt).astype(bf16),
            "wpT": np.ascontiguousarray(wpt).astype(bf16),
            "trimask": trimask,
        })
    return in_maps


def run(x, W_qkv, W_proj, trace=False):
    """Returns (y_full [4,2048,1024], BassKernelResults)."""
    from concourse import bass_utils

    nc = _get_module()
    in_maps = make_in_maps(x, W_qkv, W_proj)
    res = bass_utils.run_bass_kernel_spmd(
        nc, in_maps, core_ids=list(range(N_CORES)), trace=trace)
    y = np.zeros((4, T, 1024), np.float32)
    for b in range(4):
        y[b] = res.results[2 * b]["y"] + res.results[2 * b + 1]["y"]
    return y, res


def kernel(x, W_qkv, W_proj):
    y, _ = run(x, W_qkv, W_proj, trace=False)
    return y


# revision 33
# speedup vs baseline: 1.0221x; 1.0179x over previous
"""Causal self-attention (B=4, T=2048, C=1024, H=16, D=64) on 8 TRN2 NeuronCores.

Sharding: 4 batches x 2 head-groups (8 heads each). Core c handles batch c//2,
heads 8*(c%2) .. 8*(c%2)+7. Host pre-transposes x and slices/transposes the
weights (converted to bf16) so the device kernel needs no on-chip transposes:

  phase 1:  qkT[feat, T] = Wqk_g @ x^T   (features on partitions, t-chunk outer
            with coarse chained DMA admission in need order)
            V[t, vfeat]  = x @ Wv_g^T    (keys on partitions, + ones column)
            + the first q-chunk's attention duos interleaved between rounds,
            which spreads the exp load into the otherwise-idle ACT engine
  phase 2:  per head-pair interleaved: S^T[k, q] = K_h Q_h^T (bf16, k on
            partitions), P^T = exp(S^T/8) * causal masks (bf16),
            out^T[d|sum, q] = [V_h|1]^T P^T; normalization fused: DVE
            approx-reciprocal of the sum row (staged through SBUF), gpsimd
            partition-broadcast of 1/sum (library preloaded in phase 1), one
            tensor_mul reads PSUM and writes normalized bf16 attn^T
  phase 3:  y = attn^T.T @ Wp_g^T, interleaved between pairs as PE filler

All matmul inputs are bf16 (PSUM accumulation stays fp32), which enables
fast-weight-load and halves HBM/DVE traffic. PSUM: phase 1 holds 4 banks for
the projections + 2+2 for qc0's scores/outputs; afterwards the freed banks
double the score/output rotation. Each core returns a [2048, 1024] fp32
partial; the host sums the two head-group partials per batch.
"""

import numpy as np

T = 2048
N_CORES = 8

_CACHE = {}


def _build_module():
    from contextlib import ExitStack, nullcontext

    import concourse.tile as tile
    from concourse.tile_rust import add_dep_helper
    from concourse import bacc, mybir

    f32 = mybir.dt.float32
    bf16 = mybir.dt.bfloat16
    Exp = mybir.ActivationFunctionType.Exp
    Copy = mybir.ActivationFunctionType.Copy

    nc = bacc.Bacc("TRN2", target_bir_lowering=False, debug=False,
                   num_devices=N_CORES)

    # host-packed layouts: every DMA reads a fully contiguous DRAM range
    # (the naive [ci_full, f] layouts only give 256B-1KB runs -> ~15GB/s)
    xT_d = nc.dram_tensor("xT", (4, 128, 8 * 512), bf16, kind="ExternalInput").ap()
    wqkT_d = nc.dram_tensor("wqkT", (8, 128, 8 * 128), bf16,
                            kind="ExternalInput").ap()
    wvT_d = nc.dram_tensor("wvT", (128, 8, 512), bf16, kind="ExternalInput").ap()
    wpT_d = nc.dram_tensor("wpT", (128, 4, 1024), bf16, kind="ExternalInput").ap()
    mk_d = nc.dram_tensor("trimask", (128, 128), bf16, kind="ExternalInput").ap()
    y_d = nc.dram_tensor("y", (2048, 1024), f32, kind="ExternalOutput").ap()

    with tile.TileContext(nc) as tc, ExitStack() as ctx:
        pers = ctx.enter_context(tc.tile_pool(name="pers", bufs=1))
        sb_qT = pers.tile([128, 4, 2048], bf16, name="sb_qT")
        pq_tiles = [pers.tile([128, 512], bf16, name=f"pq{i}") for i in range(12)]
        sb_kT = pers.tile([128, 4, 2048], bf16, name="sb_kT")
        sb_v = pers.tile([128, 16, 520], bf16, name="sb_v")
        v_view = sb_v[:].rearrange("p t (h e) -> p t h e", e=65)
        sb_wpT = pers.tile([128, 4, 1024], bf16, name="sb_wpT")
        sb_trimask = pers.tile([128, 128], bf16, name="sb_trimask")
        sb_attnT = pers.tile([128, 4, 2048], bf16, name="sb_attnT")
        gp_src = pers.tile([1, 8], f32, name="gp_src")
        gp_dst = pers.tile([128, 8], f32, name="gp_dst")

        # pq0/1/4/5/8..11 serve even heads (zero band 64:128); pq2/3/6/7 odd
        PQ_EVEN = (0, 1, 4, 5, 8, 9, 10, 11)
        for i in range(12):
            o0, o1 = (64, 128) if i in PQ_EVEN else (0, 64)
            nc.vector.memset(pq_tiles[i][o0:o1, :], 0.0)
        # dummy broadcast: pulls the gpsimd library load (~7us) into phase 1
        # so the first softmax normalization doesn't pay it
        nc.vector.memset(gp_src[:], 0.0)
        nc.gpsimd.partition_broadcast(gp_dst[:], gp_src[:])

        # phase-2 pools created up front so qc0 attention can interleave
        # into phase 1. PSUM: 2 (scores) + 2 (attn out) + 4 (phase-1) banks;
        # after phase 1 the freed 4 banks become the B-side score/out pools.
        ps_sA = ctx.enter_context(tc.tile_pool(name="ps_sA", bufs=1,
                                               space="PSUM"))
        ps_oA = ctx.enter_context(tc.tile_pool(name="ps_oA", bufs=2,
                                               space="PSUM"))
        ps_s_pools = [ps_sA]
        ps_o_pools = [ps_oA]
        exp_pool = ctx.enter_context(tc.tile_pool(name="expp", bufs=8))
        norm_pool = ctx.enter_context(tc.tile_pool(name="normp", bufs=4))
        y_pool = ctx.enter_context(tc.tile_pool(name="yp", bufs=4))

        pss_n = [0]
        pso_n = [0]

        def alloc_pss():
            pool = ps_s_pools[pss_n[0] % len(ps_s_pools)]
            pss_n[0] += 1
            return pool.tile([128, 2, 512], f32, tag="pss",
                             name=f"pss{pss_n[0]}")

        def alloc_pso(shape):
            pool = ps_o_pools[pso_n[0] % len(ps_o_pools)]
            pso_n[0] += 1
            return pool.tile(shape, f32, tag="pso", name=f"pso{pso_n[0]}")

        def emit_proj(tblk, alt=False):
            for n in range(2):
                ysb = y_pool.tile([128, 512], f32, tag="ysb")
                pj = alloc_pso([128, 512])
                for ko in range(4):
                    nc.tensor.matmul(
                        pj[:],
                        lhsT=sb_attnT[:, ko, tblk * 128:(tblk + 1) * 128],
                        rhs=sb_wpT[:, ko, n * 512:(n + 1) * 512],
                        start=(ko == 0), stop=(ko == 3),
                    )
                if alt and n == 0:
                    nc.scalar.activation(ysb[:], pj[:], Copy)
                else:
                    nc.vector.tensor_copy(ysb[:], pj[:])
                nc.sync.dma_start(
                    y_d[tblk * 128:(tblk + 1) * 128, n * 512:(n + 1) * 512],
                    ysb[:])

        def norm_store(po, rr, p_, qc, tail=False):
            # normalize: sum row -> SBUF (recip from PSUM is broken on HW),
            # approx-reciprocal, gpsimd broadcast, then one fused mul reads
            # the PSUM attn rows and writes normalized bf16 attn^T.
            # Off the tail, run at low priority so the next duo's mask muls
            # win the DVE queue (a norm-first ordering stalls PE ~2us).
            att_slice = sb_attnT[rr:rr + 64, p_, qc * 512:(qc + 1) * 512]
            prio = (tc.high_priority(offset=300) if tail
                    else tc.high_priority(offset=-150))
            sums = norm_pool.tile([1, 512], f32, tag="sums")
            recip = norm_pool.tile([1, 512], f32, tag="recip")
            bcast = norm_pool.tile([128, 512], f32, tag="bcast")
            with prio:
                if tail:
                    # ACT is idle at the tail; free the DVE for the recip
                    nc.scalar.activation(sums[:], po[64:65, :], Copy)
                else:
                    nc.vector.tensor_copy(sums[:], po[64:65, :])
                nc.vector.reciprocal_approx_fast(out=recip[:], in_=sums[:])
                nc.gpsimd.partition_broadcast(bcast[:], recip[:])
                nc.vector.tensor_mul(att_slice, po[0:64, :], bcast[0:64, :])

        DUOS = ((0, 2), (4, 6), (1, 3), (5, 7))

        def emit_duo(qc, di, mid=None, proj=True):
            nblk = 4 * qc + 4
            hA, hB = DUOS[di]
            rr = (hA % 2) * 64
            poA = alloc_pso([65, 512])
            poB = alloc_pso([65, 512])
            if qc == 1 and di in (0, 1):
                pqA, pqB = pq_tiles[8 + 2 * di], pq_tiles[9 + 2 * di]
            else:
                base = (0, 4, 2, 6)[di]
                pqA, pqB = pq_tiles[base], pq_tiles[base + 1]
            duo = ((hA, poA, pqA), (hB, poB, pqB))
            if qc > 0 and not (qc == 1 and di in (0, 1)):
                for h, po, pq in duo:
                    nc.vector.tensor_copy(
                        pq[rr:rr + 64, :],
                        sb_qT[rr:rr + 64, h // 2, qc * 512:(qc + 1) * 512])
            for jg in range(nblk // 2):
                # all scores of the duo first: the partner's scores hide
                # each head's exp latency before its PV consumes it.
                # Diagonal pairs skip the below-diagonal score columns
                # (never read); the i01 pair swaps PSUM slots so the live
                # region is one contiguous flat range for a single exp.
                diag0 = jg * 2 == 4 * qc        # blocks i=0,1
                diag2 = jg * 2 == 4 * qc + 2    # blocks i=2,3
                ets = []
                for h, po, pq in duo:
                    p_ = h // 2
                    pss = alloc_pss()
                    for jj in range(2):
                        j = jg * 2 + jj
                        slot = 1 - jj if diag0 else jj
                        ls = 128 * jj if diag0 else (256 if diag2 else 0)
                        nc.tensor.matmul(
                            pss[:, slot, ls:512],
                            lhsT=sb_kT[:, p_, j * 128:(j + 1) * 128],
                            rhs=pq[:, ls:512],
                            start=True, stop=True,
                        )
                    et = exp_pool.tile([128, 2, 512], bf16, tag="expT")
                    if diag2:
                        nc.scalar.activation(et[:, :, 256:512],
                                             pss[:, :, 256:512],
                                             Exp, scale=0.125)
                    elif diag0:
                        ef = et[:].rearrange("p a b -> p (a b)")
                        pf = pss[:].rearrange("p a b -> p (a b)")
                        nc.scalar.activation(ef[:, 128:1024], pf[:, 128:1024],
                                             Exp, scale=0.125)
                    else:
                        nc.scalar.activation(et[:], pss[:], Exp, scale=0.125)
                    ets.append(et)
                for (h, po, pq), et in zip(duo, ets):
                    for jj in range(2):
                        j = jg * 2 + jj
                        slot = 1 - jj if diag0 else jj
                        lo = 0
                        if j >= 4 * qc:
                            i = j - 4 * qc
                            lo = i * 128
                            nc.vector.tensor_mul(
                                et[:, slot, lo:lo + 128],
                                et[:, slot, lo:lo + 128], sb_trimask[:])
                        nc.tensor.matmul(
                            po[:, lo:512],
                            lhsT=v_view[:, j, h, :],
                            rhs=et[:, slot, lo:512],
                            start=(j == 0), stop=(j == nblk - 1),
                        )
            if mid is not None:
                mid()
            for h, po, pq in duo:
                norm_store(po, (h % 2) * 64, h // 2, qc,
                           tail=(qc == 3 and di == 3))
            if qc > 0 and proj:
                emit_proj((qc - 1) * 4 + di)

        # ---------------- phase 1: qkv projections (+ qc0 attention) -------
        with ExitStack() as p1:
            ph1 = p1.enter_context(tc.tile_pool(name="ph1", bufs=1))
            ps_big = p1.enter_context(tc.tile_pool(name="ps_big", bufs=4,
                                                   space="PSUM"))
            wqk_pool = p1.enter_context(tc.tile_pool(name="wqk", bufs=8))
            xt_pool = p1.enter_context(tc.tile_pool(name="xt", bufs=2))

            wqkT_r = wqkT_d.rearrange("fb ci (co f) -> fb ci co f", f=128)
            xT_r = xT_d.rearrange("tc ci (co t) -> tc ci co t", t=512)

            chain_prev = [None]

            def chain(d):
                if chain_prev[0] is not None:
                    add_dep_helper(d.ins, chain_prev[0].ins, sync=False,
                                   reason="adm chain")
                chain_prev[0] = d
                return d

            # K features first, then Q
            fbs = [4, 5, 6, 7, 0, 1, 2, 3]
            wtiles = {fb: wqk_pool.tile([128, 8, 128], bf16, tag="wqk",
                                        name=f"wt{fb}") for fb in fbs}
            sb_wvT = ph1.tile([128, 8, 512], bf16, name="sb_wvT")
            xchunk = xt_pool.tile([128, 8, 512], bf16, tag="xt", name="xc0")
            # admission order = need order (descriptors round-robin across the
            # 16 DMA engines in trigger order, so early triggers finish first)
            chain(nc.sync.dma_start(wtiles[4][:], wqkT_r[4]))
            chain(nc.sync.dma_start(xchunk[:, 0:4, :], xT_r[0, :, 0:4, :]))
            chain(nc.sync.dma_start(wtiles[5][:], wqkT_r[5]))
            chain(nc.sync.dma_start(xchunk[:, 4:8, :], xT_r[0, :, 4:8, :]))
            for fb in (6, 7):
                chain(nc.sync.dma_start(wtiles[fb][:], wqkT_r[fb]))
            chain(nc.sync.dma_start(sb_trimask[:], mk_d[:]))
            for fb in (0, 1, 2, 3):
                chain(nc.sync.dma_start(wtiles[fb][:], wqkT_r[fb]))
            chain(nc.sync.dma_start(sb_wvT[:], wvT_d[:]))
            chain(nc.sync.dma_start(sb_wpT[:], wpT_d[:]))

            # ones column of sb_v via exp(0)=1 — also warms the ACT exp table
            zeros = ph1.tile([128, 128], f32, name="zeros")
            nc.vector.memset(zeros[:], 0.0)
            nc.scalar.activation(
                v_view[:, :, :, 64:65],
                zeros[:].rearrange("p (a b c) -> p a b c", a=16, b=8),
                Exp,
            )

            for tci in range(4):
                if tci < 3:
                    nxt = xt_pool.tile([128, 8, 512], bf16, tag="xt",
                                       name=f"xc{tci + 1}")
                    chain(nc.sync.dma_start(nxt[:], xT_r[tci + 1]))
                for fb in fbs:
                    dst, pblk = (sb_kT, fb - 4) if fb >= 4 else (sb_qT, fb)
                    ps = ps_big.tile([128, 512], f32, tag="psb")
                    for co in range(8):
                        nc.tensor.matmul(
                            ps[:],
                            lhsT=wtiles[fb][:, co, :],
                            rhs=xchunk[:, co, :],
                            start=(co == 0), stop=(co == 7),
                        )
                    if tci >= 2 and fb % 2 == 0:
                        nc.scalar.activation(
                            dst[:, pblk, tci * 512:(tci + 1) * 512], ps[:],
                            Copy)
                    else:
                        nc.vector.tensor_copy(
                            dst[:, pblk, tci * 512:(tci + 1) * 512], ps[:])
                    if tci == 0 and fb == 3:
                        # prefill padded-q for all of qc0
                        for pi, h in ((0, 0), (1, 2), (4, 4), (5, 6),
                                      (2, 1), (3, 3), (6, 5), (7, 7)):
                            rr0 = (h % 2) * 64
                            nc.vector.tensor_copy(
                                pq_tiles[pi][rr0:rr0 + 64, :],
                                sb_qT[rr0:rr0 + 64, h // 2, 0:512])
                    if tci == 1 and fb == 3:
                        # prefill qc1's even duos too (ramp relief)
                        for pi, h in ((8, 0), (9, 2), (10, 4), (11, 6)):
                            nc.vector.tensor_copy(
                                pq_tiles[pi][0:64, :],
                                sb_qT[0:64, h // 2, 512:1024])
                if tci == 2:
                    emit_duo(0, 1)
                elif tci == 3:
                    emit_duo(0, 3)
                for tb in range(4):
                    tblk = tci * 4 + tb
                    ps = ps_big.tile([128, 512], f32, tag="psb")
                    for co in range(8):
                        nc.tensor.matmul(
                            ps[:],
                            lhsT=xchunk[:, co, tb * 128:(tb + 1) * 128],
                            rhs=sb_wvT[:, co, :],
                            start=(co == 0), stop=(co == 7),
                        )
                    lowprio = (tc.high_priority(offset=-400) if tci == 3
                               else nullcontext())
                    with lowprio:
                        if tci >= 2 and tb % 2 == 0:
                            nc.scalar.activation(
                                v_view[:, tblk, :, 0:64],
                                ps[:].rearrange("p (h d) -> p h d", d=64),
                                Copy)
                        else:
                            nc.vector.tensor_copy(
                                v_view[:, tblk, :, 0:64],
                                ps[:].rearrange("p (h d) -> p h d", d=64),
                            )
                if tci == 1:
                    emit_duo(0, 0)
                elif tci == 2:
                    emit_duo(0, 2)
                if tci < 3:
                    xchunk = nxt

        # ---------------- phase 2 + 3: attention + proj ----------------
        # phase 1's 4 PSUM banks are free now: add the B-side pools so the
        # score/out rotations double (allocators alternate A/B)
        ps_s_pools.append(ctx.enter_context(
            tc.tile_pool(name="ps_sB", bufs=1, space="PSUM")))
        ps_o_pools.append(ctx.enter_context(
            tc.tile_pool(name="ps_oB", bufs=2, space="PSUM")))

        for qc in range(1, 4):
            # duo interleave: second head fills the first head's exp latency
            for di in range(4):
                if qc == 3 and di == 3:
                    # keep PE fed while the last duo's norm chain drains:
                    # tblks 12,13 run through the freed score-PSUM rings
                    # ([128,2,512] = both column halves); their ko0/ko1
                    # matmuls (heads 0-3, long ready) fill the gap, ko2/ko3
                    # follow after the norm
                    fpjs = []

                    def mid():
                        for tblk in (12, 13):
                            pj = alloc_pss()
                            for n in range(2):
                                for ko in (0, 1):
                                    nc.tensor.matmul(
                                        pj[:, n, :],
                                        lhsT=sb_attnT[:, ko,
                                                      tblk * 128:(tblk + 1) * 128],
                                        rhs=sb_wpT[:, ko,
                                                   n * 512:(n + 1) * 512],
                                        start=(ko == 0), stop=False,
                                    )
                            fpjs.append((tblk, pj))

                    emit_duo(3, 3, mid=mid)
                    for tblk, pj in fpjs:
                        for n in range(2):
                            for ko in (2, 3):
                                nc.tensor.matmul(
                                    pj[:, n, :],
                                    lhsT=sb_attnT[:, ko,
                                                  tblk * 128:(tblk + 1) * 128],
                                    rhs=sb_wpT[:, ko, n * 512:(n + 1) * 512],
                                    start=False, stop=(ko == 3),
                                )
                            ysb = y_pool.tile([128, 512], f32, tag="ysb")
                            if n == 0:
                                nc.scalar.activation(ysb[:], pj[:, n, :], Copy)
                            else:
                                nc.vector.tensor_copy(ysb[:], pj[:, n, :])
                            nc.sync.dma_start(
                                y_d[tblk * 128:(tblk + 1) * 128,
                                    n * 512:(n + 1) * 512],
                                ysb[:])
                else:
                    emit_duo(qc, di)
            if qc == 3:
                for tblk in range(14, 16):
                    emit_proj(tblk, alt=True)

    nc.compile()
    return nc


def _get_module():
    if "nc" not in _CACHE:
        _CACHE["nc"] = _build_module()
    return _CACHE["nc"]


def _make_trimask():
    # trimask[kk, q] = 1 iff q >= kk (diagonal 128x128 block)
    q = np.arange(128)[None, :]
    kk = np.arange(128)[:, None]
    return (q >= kk).astype(np.float32)


def make_in_maps(x, W_qkv, W_proj):
    import ml_dtypes

    bf16 = ml_dtypes.bfloat16
    x = np.asarray(x, dtype=np.float32)
    W_qkv = np.asarray(W_qkv, dtype=np.float32)
    W_proj = np.asarray(W_proj, dtype=np.float32)
    trimask = _make_trimask().astype(bf16)
    in_maps = []
    for c in range(N_CORES):
        b, g = c // 2, c % 2
        s = 512 * g
        wqk = np.concatenate([W_qkv[s:s + 512], W_qkv[1024 + s:1024 + s + 512]], 0)
        # packed layouts (see _build_module): contiguous per DMA
        xt = x[b].T.reshape(8, 128, 4, 512).transpose(2, 1, 0, 3)   # tc ci co t
        wqkt = wqk.T.reshape(8, 128, 8, 128).transpose(2, 1, 0, 3)  # fb ci co f
        wvt = W_qkv[2048 + s:2048 + s + 512].T.reshape(8, 128, 512)
        wvt = wvt.transpose(1, 0, 2)                                # ci co f
        wpt = W_proj[:, s:s + 512].T.reshape(4, 128, 1024)
        wpt = wpt.transpose(1, 0, 2)                                # ki ko n
        in_maps.append({
            "xT": np.ascontiguousarray(xt.reshape(4, 128, 8 * 512)).astype(bf16),
            "wqkT": np.ascontiguousarray(wqkt.reshape(8, 128, 8 * 128)).astype(bf16),
            "wvT": np.ascontiguousarray(wvt).astype(bf16),
            "wpT": np.ascontiguousarray(wpt).astype(bf16),
            "trimask": trimask,
        })
    return in_maps


def run(x, W_qkv, W_proj, trace=False):
    """Returns (y_full [4,2048,1024], BassKernelResults)."""
    from concourse import bass_utils

    nc = _get_module()
    in_maps = make_in_maps(x, W_qkv, W_proj)
    res = bass_utils.run_bass_kernel_spmd(
        nc, in_maps, core_ids=list(range(N_CORES)), trace=trace)
    y = np.zeros((4, T, 1024), np.float32)
    for b in range(4):
        y[b] = res.results[2 * b]["y"] + res.results[2 * b + 1]["y"]
    return y, res


def kernel(x, W_qkv, W_proj):
    y, _ = run(x, W_qkv, W_proj, trace=False)
    return y
